# revision 1
# baseline (speedup 1.0000x reference)
"""NestedGIN (4-layer GIN + 2-level pooling + MLP head) on 8 Trainium2 NeuronCores.

Strategy:
  - Nodes (and their incident in-edges, i.e. edges grouped by dst) are sharded
    across 8 cores; MLP weights are replicated.
  - Per layer: each core gathers h[src] for its edges from a replicated
    node-major copy of h in HBM (hardware dma_gather, 256B rows), then
    scatter-adds into its node slice via one-hot matmuls on the TensorEngine
    (edges sorted by dst; 128-edge tiles vs 128-node windows; psum窗 flushed
    into a feature-major SBUF accumulator at data-driven dynamic offsets).
  - The GIN MLP runs feature-major ([64, nodes] tiles, weights stationary).
  - The updated slice is transposed back to node-major, exported to HBM and
    AllGather'ed for the next layer's gathers.
  - Final: subgraph pooling (one-hot matmul vs node_to_subgraph) -> AllReduce
    of partial subgraph sums -> graph pooling -> MLP head -> log_softmax.

Host-side numpy does only index/layout prep (sharding, sorting, padding) plus
the layer-0 input-feature reindex x[src] (pure copy, no arithmetic).
"""

import os
import sys

for _p in ("/opt/trn_rl_repo", "/opt/pypackages"):
    if os.path.isdir(_p) and _p not in sys.path:
        sys.path.append(_p)

import numpy as np

import concourse.bass as bass
import concourse.bacc as bacc
import concourse.tile as tile
import concourse.mybir as mybir

F32 = mybir.dt.float32
I32 = mybir.dt.int32
I16 = mybir.dt.int16


class Cfg:
    def __init__(self, N=100000, E=1600000, S=2000, NGRAPH=64, OUT=8,
                 CORES=8, CHUNK=25000):
        self.N, self.E, self.S = N, S and N and E, S  # keep E explicit below
        self.E = E
        self.NGRAPH, self.OUT, self.CORES = NGRAPH, OUT, CORES
        self.H = 64
        self.NPC = N // CORES                      # nodes per core
        assert N % CORES == 0
        self.CHUNK = CHUNK                         # gather-source chunk rows (int16 idx limit)
        assert CHUNK <= 32767
        self.NCH = (N + CHUNK - 1) // CHUNK        # chunks
        assert N % CHUNK == 0
        self.GT = 4                                # edge tiles (128 edges) per group
        self.BG = 8                                # groups per dma_gather batch
        self.W = 160                               # node window per group
        self.NPC_PAD = ((self.NPC + 511) // 512) * 512
        self.NMG = self.NPC_PAD // 512             # MLP node-groups of 512
        self.NT_REAL = (self.NPC + 127) // 128     # node-major tiles (transpose/export)
        self.LAST_ROWS = self.NPC - (self.NT_REAL - 1) * 128
        self.SPAD = 2048                           # padded #subgraphs
        assert S <= self.SPAD
        self.SWIN = 512                            # per-core subgraph window
        self.S_T = self.SPAD // 128
        assert NGRAPH <= 64


# ----------------------------------------------------------------------------
# Host-side prep: shard + sort edges, build tiles/groups, pack device inputs.
# ----------------------------------------------------------------------------

def _prep(inputs, C: Cfg):
    x = np.asarray(inputs["x"], np.float32).reshape(C.N)
    ei = np.asarray(inputs["edge_index"]).astype(np.int64)
    n2s = np.asarray(inputs["node_to_subgraph"]).astype(np.int64)
    s2g = np.asarray(inputs["subgraph_to_graph"]).astype(np.int64)
    src_all, dst_all = ei[0], ei[1]

    # -------- per-core edge groups --------
    per_core = []  # per core: list over chunks of list of groups
    for c in range(C.CORES):
        lo = c * C.NPC
        m = (dst_all >= lo) & (dst_all < lo + C.NPC)
        s, d = src_all[m], dst_all[m]
        ch = s // C.CHUNK
        o = np.lexsort((s, d, ch))
        s, d, ch = s[o], d[o], ch[o]
        chunks = []
        for cv in range(C.NCH):
            sel = ch == cv
            ss, dd = s[sel], d[sel]
            groups = []
            i, n = 0, len(ss)
            cap = C.GT * 128
            while i < n:
                j = min(i + cap, n)
                # keep node span < W
                if dd[j - 1] - dd[i] >= C.W:
                    j = int(np.searchsorted(dd, dd[i] + C.W, side="left"))
                base = int(dd[i])
                gs = ss[i:j]
                gd = dd[i:j] - base
                pad = cap - (j - i)
                if pad:
                    gs = np.concatenate([gs, np.zeros(pad, np.int64) + cv * C.CHUNK])
                    gd = np.concatenate([gd, np.full(pad, -1, np.int64)])
                groups.append((base - lo, gs, gd))
                i = j
            chunks.append(groups)
        per_core.append(chunks)

    # uniform group counts per chunk across cores (SPMD)
    NG = []
    for cv in range(C.NCH):
        mx = max(len(per_core[c][cv]) for c in range(C.CORES))
        mx = ((mx + C.BG - 1) // C.BG) * C.BG
        NG.append(mx)
    NGTOT = sum(NG)
    NBTOT = NGTOT // C.BG
    cap = C.GT * 128

    pad_group = (C.NPC, np.zeros(cap, np.int64), np.full(cap, -1, np.int64))

    in_maps = []
    for c in range(C.CORES):
        lo = c * C.NPC
        dstl = np.empty((128, NGTOT * C.GT), np.float32)
        xsrc = np.zeros((128, NGTOT * C.GT), np.float32)
        flush = np.empty((1, NGTOT), np.int32)
        bcols = C.BG * cap // 16
        idx = np.empty((16, NBTOT * bcols), np.int16)  # replicated to 128 below
        g_i = 0
        for cv in range(C.NCH):
            groups = per_core[c][cv]
            for k in range(NG[cv]):
                off, gs, gd = groups[k] if k < len(groups) else \
                    (pad_group[0], pad_group[1] + cv * C.CHUNK, pad_group[2])
                flush[0, g_i] = off
                dstl[:, g_i * C.GT:(g_i + 1) * C.GT] = \
                    gd.reshape(C.GT, 128).T.astype(np.float32)
                xv = x[gs]
                xv[gd < 0] = 0.0
                xsrc[:, g_i * C.GT:(g_i + 1) * C.GT] = xv.reshape(C.GT, 128).T
                # wrapped int16 idx layout for this group's slot of its batch
                loc = (gs - cv * C.CHUNK).astype(np.int16)
                b = g_i // C.BG
                half = g_i % C.BG
                gcols = cap // 16
                blk = loc.reshape(gcols, 16).T
                idx[:, b * bcols + half * gcols:b * bcols + (half + 1) * gcols] = blk
                g_i += 1
        assert g_i == NGTOT

        own_n2s = n2s[lo:lo + C.NPC]
        sstart = int(min(max(0, own_n2s.min()), C.SPAD - C.SWIN))
        assert own_n2s.max() - sstart < C.SWIN, "subgraph window overflow"
        n2s_lin = np.full(C.NT_REAL * 128, -1, np.float32)
        n2s_lin[:C.NPC] = own_n2s - sstart
        n2s_loc = n2s_lin.reshape(C.NT_REAL, 128).T

        s2g_lin = np.full(C.SPAD, -1, np.float32)
        s2g_lin[:C.S] = s2g
        s2g_loc = s2g_lin.reshape(C.S_T, 128).T

        xown = np.zeros((1, C.NPC_PAD), np.float32)
        xown[0, :C.NPC] = x[lo:lo + C.NPC]

        m = {
            "g_idx": np.tile(idx, (8, 1)), "g_dstl": dstl, "g_xsrc": xsrc, "g_flush": flush,
            "xown": xown, "n2s": n2s_loc.astype(np.float32),
            "s2g": s2g_loc.astype(np.float32),
            "sstart": np.array([[sstart]], np.int32),
            "w1a": np.asarray(inputs["conv1_w1"], np.float32),
            "b1a": np.asarray(inputs["conv1_b1"], np.float32).reshape(C.H, 1),
            "w2a": np.asarray(inputs["conv1_w2"], np.float32),
            "b2a": np.asarray(inputs["conv1_b2"], np.float32).reshape(C.H, 1),
            "cw1": np.concatenate(list(np.asarray(inputs["convs_w1"], np.float32)), axis=1),
            "cb1": np.asarray(inputs["convs_b1"], np.float32).T.copy(),
            "cw2": np.concatenate(list(np.asarray(inputs["convs_w2"], np.float32)), axis=1),
            "cb2": np.asarray(inputs["convs_b2"], np.float32).T.copy(),
            "l1w": np.asarray(inputs["lin1_w"], np.float32),
            "l1b": np.asarray(inputs["lin1_b"], np.float32).reshape(C.H, 1),
            "l2w": np.asarray(inputs["lin2_w"], np.float32),
            "l2b": np.asarray(inputs["lin2_b"], np.float32).reshape(C.OUT, 1),
            "iota": np.tile(np.arange(512, dtype=np.float32), (128, 1)),
            "ident": np.eye(128, dtype=np.float32),
        }
        in_maps.append(m)

    counts = {"NG": NG, "NGTOT": NGTOT, "NBTOT": NBTOT}
    return in_maps, counts


# ----------------------------------------------------------------------------
# Device module
# ----------------------------------------------------------------------------

def _build(C: Cfg, counts, repeat=1):
    ABL = set(os.environ.get("KABL", "").split(","))
    from concourse.tile import add_dep_helper

    NG, NGTOT, NBTOT = counts["NG"], counts["NGTOT"], counts["NBTOT"]
    NL = 4  # GIN layers
    DVE = mybir.EngineType.DVE
    AF = mybir.ActivationFunctionType
    AL = mybir.AluOpType
    BCOLS = C.BG * C.GT * 128 // 16     # idx cols per gather batch
    GCOLS = C.BG * C.GT                 # dstl/xsrc cols per batch
    NIDX = C.BG * C.GT * 128            # idxs per gather

    LOCAL = os.environ.get("KLOCAL") == "1"
    nc = bacc.Bacc("TRN2", target_bir_lowering=False, debug=False,
                   enable_asserts=False,
                   num_devices=1 if LOCAL else C.CORES)

    def ein(name, shape, dt=F32):
        return nc.dram_tensor(name, shape, dt, kind="ExternalInput").ap()

    IDX = ein("g_idx", [128, NBTOT * BCOLS], I16)
    DSTL = ein("g_dstl", [128, NGTOT * C.GT])
    XSRC = ein("g_xsrc", [128, NGTOT * C.GT])
    FLUSH = ein("g_flush", [1, NGTOT], I32)
    XOWN = ein("xown", [1, C.NPC_PAD])
    N2S = ein("n2s", [128, C.NT_REAL])
    S2G = ein("s2g", [128, C.S_T])
    SSTART = ein("sstart", [1, 1], I32)
    W1A, B1A = ein("w1a", [1, C.H]), ein("b1a", [C.H, 1])
    W2A, B2A = ein("w2a", [C.H, C.H]), ein("b2a", [C.H, 1])
    CW1, CB1 = ein("cw1", [C.H, 3 * C.H]), ein("cb1", [C.H, 3])
    CW2, CB2 = ein("cw2", [C.H, 3 * C.H]), ein("cb2", [C.H, 3])
    L1W, L1B = ein("l1w", [C.H, C.H]), ein("l1b", [C.H, 1])
    L2W, L2B = ein("l2w", [C.H, C.OUT]), ein("l2b", [C.OUT, 1])
    IOTA = ein("iota", [128, 512])
    IDENT = ein("ident", [128, 128])
    OUTT = nc.dram_tensor("out", [C.NGRAPH, C.OUT], F32, kind="ExternalOutput").ap()

    hbuf = [nc.dram_tensor(f"hbuf{k}", [C.N, C.H], F32, kind="Internal",
                           addr_space="Shared").ap() for k in range(2)]
    agin = [nc.dram_tensor(f"agin{k}", [C.NPC, C.H], F32, kind="Internal").ap()
            for k in range(2)]
    ppin = nc.dram_tensor("ppin", [C.H, C.SPAD], F32, kind="Internal").ap()
    ppout = nc.dram_tensor("ppout", [C.H, C.SPAD], F32, kind="Internal",
                           addr_space="Shared").ap()

    RG = [list(range(C.CORES))]
    AGGW = C.NPC_PAD + C.W

    with tile.TileContext(nc) as tc:
        from concourse import library_config
        nc.gpsimd.load_library(library_config.mlp)
        with (
            tc.tile_pool(name="const", bufs=1) as P0,
            tc.tile_pool(name="stream", bufs=3) as PS,
            tc.tile_pool(name="msgs", bufs=2) as PM,
            tc.tile_pool(name="oh", bufs=4) as PO,
            tc.tile_pool(name="mlp", bufs=3) as PL,
            tc.tile_pool(name="ps_sc", bufs=3, space="PSUM") as PSC,
            tc.tile_pool(name="ps_mlp", bufs=2, space="PSUM") as PSM,
            tc.tile_pool(name="ps_tr", bufs=2, space="PSUM") as PST,
            tc.tile_pool(name="ps_pool", bufs=1, space="PSUM") as PSP,
        ):
            # ---- resident tiles ----
            def load(ap, shape, dt=F32, tag=None):
                t = P0.tile(shape, dt, tag=tag)
                nc.sync.dma_start(t[:], ap)
                return t

            flush_sb = load(FLUSH, [1, NGTOT], I32, tag="c_flush")
            n2s_sb = load(N2S, [128, C.NT_REAL], tag="c_n2s")
            s2g_sb = load(S2G, [128, C.S_T], tag="c_s2g")
            sstart_sb = load(SSTART, [1, 1], I32, tag="c_sstart")
            w1a_sb, b1a_sb = load(W1A, [1, C.H], tag="c_w1a"), load(B1A, [C.H, 1], tag="c_b1a")
            w2a_sb, b2a_sb = load(W2A, [C.H, C.H], tag="c_w2a"), load(B2A, [C.H, 1], tag="c_b2a")
            cw1_sb, cb1_sb = load(CW1, [C.H, 3 * C.H], tag="c_cw1"), load(CB1, [C.H, 3], tag="c_cb1")
            cw2_sb, cb2_sb = load(CW2, [C.H, 3 * C.H], tag="c_cw2"), load(CB2, [C.H, 3], tag="c_cb2")
            l1w_sb, l1b_sb = load(L1W, [C.H, C.H], tag="c_l1w"), load(L1B, [C.H, 1], tag="c_l1b")
            l2w_sb, l2b_sb = load(L2W, [C.H, C.OUT], tag="c_l2w"), load(L2B, [C.OUT, 1], tag="c_l2b")
            iota_sb = load(IOTA, [128, 512], tag="c_iota")
            ident_sb = load(IDENT, [128, 128], tag="c_ident")

            agg = P0.tile([C.H, AGGW], F32, tag="agg")
            hT = P0.tile([C.H, C.NPC_PAD], F32, tag="hT")
            expb = P0.tile([128, C.NT_REAL * C.H], F32, tag="expb")
            pp_sb = P0.tile([C.H, C.SPAD], F32, tag="pp_sb")
            p_sb = P0.tile([C.H, C.SPAD], F32, tag="p_sb")

            regs = [nc.alloc_registers(f"rof{k}", engines=[DVE]) for k in range(4)]
            sreg = nc.alloc_registers("sreg", engines=[DVE])

            ag_inst = None  # last AllGather instruction (DRAM dep anchor)

            for _rep in range(repeat):
              for layer in range(NL):
                  l0 = layer == 0
                  nc.vector.memset(agg[:], 0.0)
                  if l0:
                      w1, b1, w2, b2 = w1a_sb, b1a_sb, w2a_sb, b2a_sb
                  else:
                      r0 = (layer - 1) * C.H
                      li = layer - 1
                      w1 = cw1_sb[:, r0:r0 + C.H]
                      b1 = cb1_sb[:, li:li + 1]
                      w2 = cw2_sb[:, r0:r0 + C.H]
                      b2 = cb2_sb[:, li:li + 1]
                  src_hbuf = hbuf[(layer + 1) % 2] if not l0 else None

                  # ---------- aggregation ----------
                  g_i = 0
                  b_i = 0
                  for cv in range(C.NCH):
                      view = None
                      if not l0:
                          view = src_hbuf[cv * C.CHUNK:(cv + 1) * C.CHUNK, :]
                      for b in range(NG[cv] // C.BG):
                          if l0:
                              mt = None
                              xt = PS.tile([128, GCOLS], F32, tag="xsrc_t")
                              nc.sync.dma_start(
                                  xt[:], XSRC[:, g_i * C.GT:g_i * C.GT + GCOLS])
                          else:
                              it = PS.tile([128, BCOLS], I16, tag="idx_t")
                              nc.sync.dma_start(
                                  it[:], IDX[:, b_i * BCOLS:(b_i + 1) * BCOLS])
                              mt = PM.tile([128, GCOLS, C.H], F32, tag="msgs")
                              gi = nc.gpsimd.dma_gather(
                                  mt[:], view, it[:], NIDX, NIDX, C.H,
                                  single_packet=False)
                              if ag_inst is not None:
                                  add_dep_helper(gi.ins, ag_inst.ins,
                                                 reason="gather after allgather")
                          dt_t = PS.tile([128, GCOLS], F32, tag="dstl_t")
                          nc.sync.dma_start(
                              dt_t[:], DSTL[:, g_i * C.GT:g_i * C.GT + GCOLS])
                          for gg in range(C.BG):
                              ps = PSC.tile([1 if l0 else C.H, C.W], F32, tag="ps_sc")
                              for t in range(C.GT):
                                  tcol = gg * C.GT + t
                                  oh = PO.tile([128, C.W], F32, tag="oh")
                                  nc.vector.tensor_scalar(
                                      oh[:], iota_sb[:, :C.W],
                                      dt_t[:, tcol:tcol + 1], None, AL.is_equal)
                                  lhsT = (xt[:, tcol:tcol + 1] if l0 else
                                          mt[:, tcol, :])
                                  nc.tensor.matmul(ps[:], lhsT, oh[:],
                                                   start=(t == 0), stop=(t == C.GT - 1))
                              r = regs[g_i % 4]
                              nc.vector.reg_load(r, flush_sb[0:1, g_i:g_i + 1])
                              off = nc.vector.snap(r, donate=False, min_val=0,
                                                   max_val=C.NPC)
                              dyn = agg[0:1, bass.ds(off, C.W)] if l0 else \
                                  agg[:, bass.ds(off, C.W)]
                              nc.vector.tensor_tensor(dyn, ps[:], dyn, AL.add)
                              g_i += 1
                          b_i += 1
                  assert g_i == NGTOT

                  # ---------- update (h + agg -> MLP) ----------
                  for ngp in range(C.NMG):
                      sl = slice(ngp * 512, (ngp + 1) * 512)
                      if l0:
                          xo = PL.tile([1, 512], F32, tag="xo")
                          nc.sync.dma_start(xo[:], XOWN[0:1, sl])
                          hin = PL.tile([1, 512], F32, tag="hin0")
                          nc.vector.tensor_tensor(hin[:], xo[:], agg[0:1, sl], AL.add)
                      else:
                          hin = PL.tile([C.H, 512], F32, tag="hin")
                          nc.vector.tensor_tensor(hin[:], hT[:, sl], agg[:, sl], AL.add)
                      ps1 = PSM.tile([C.H, 512], F32, tag="ps_mlp")
                      nc.tensor.matmul(ps1[:], w1, hin[:], start=True, stop=True)
                      t1 = PL.tile([C.H, 512], F32, tag="t1")
                      nc.scalar.activation(t1[:], ps1[:], AF.Relu, bias=b1)
                      ps2 = PSM.tile([C.H, 512], F32, tag="ps_mlp")
                      nc.tensor.matmul(ps2[:], w2, t1[:], start=True, stop=True)
                      nc.scalar.activation(hT[:, sl], ps2[:], AF.Relu, bias=b2)

                  # ---------- transpose to node-major ----------
                  last = layer == NL - 1
                  if last:
                      ps_s = PSP.tile([C.H, C.SWIN], F32, tag="ps_pool")
                  for j in range(C.NT_REAL):
                      pt = PST.tile([128, C.H], F32, tag="ps_tr")
                      nc.tensor.transpose(pt[:], hT[:, j * 128:(j + 1) * 128],
                                          ident_sb[:C.H, :C.H])
                      nc.scalar.activation(expb[:, j * C.H:(j + 1) * C.H], pt[:],
                                           AF.Copy)
                      if last:
                          ohs = PO.tile([128, C.SWIN], F32, tag="ohs")
                          nc.vector.tensor_scalar(ohs[:], iota_sb[:, :C.SWIN],
                                                  n2s_sb[:, j:j + 1], None,
                                                  AL.is_equal)
                          nc.tensor.matmul(ps_s[:],
                                           expb[:, j * C.H:(j + 1) * C.H],
                                           ohs[:], start=(j == 0),
                                           stop=(j == C.NT_REAL - 1))

                  if not last:
                      # export + AllGather
                      dst = agin[layer % 2]
                      nf = C.NT_REAL - 1
                      d1 = nc.sync.dma_start(
                          dst[0:nf * 128, :].rearrange("(b p) f -> p b f", p=128),
                          expb[:, :nf * C.H].rearrange("p (b f) -> p b f", f=C.H))
                      d2 = nc.sync.dma_start(
                          dst[nf * 128:C.NPC, :],
                          expb[:C.LAST_ROWS, nf * C.H:(nf + 1) * C.H])
                      if LOCAL:
                          ag = nc.sync.dma_start(
                              hbuf[layer % 2][0:C.NPC, :], dst)
                      else:
                          ag = nc.gpsimd.collective_compute(
                              "AllGather", AL.bypass, replica_groups=RG,
                              ins=[dst], outs=[hbuf[layer % 2]])
                      add_dep_helper(ag.ins, d1.ins, reason="ag after export")
                      add_dep_helper(ag.ins, d2.ins, reason="ag after export")
                      ag_inst = ag

            # ---------- subgraph partial sums -> AllReduce ----------
            nc.vector.memset(pp_sb[:], 0.0)
            nc.vector.reg_load(sreg, sstart_sb[0:1, 0:1])
            soff = nc.vector.snap(sreg, donate=True, min_val=0,
                                  max_val=C.SPAD - C.SWIN)
            dynp = pp_sb[:, bass.ds(soff, C.SWIN)]
            nc.vector.tensor_copy(dynp, ps_s[:])
            d3 = nc.sync.dma_start(ppin, pp_sb[:])
            if LOCAL:
                ar = nc.sync.dma_start(ppout, ppin)
            else:
                ar = nc.gpsimd.collective_compute(
                    "AllReduce", AL.add, replica_groups=RG, ins=[ppin], outs=[ppout])
            add_dep_helper(ar.ins, d3.ins, reason="ar after store")
            d4 = nc.sync.dma_start(p_sb[:], ppout)
            add_dep_helper(d4.ins, ar.ins, reason="load after ar")

            # ---------- graph pooling ----------
            ps_g = PSP.tile([C.H, C.NGRAPH], F32, tag="ps_pool")
            for jt in range(C.S_T):
                pt = PST.tile([128, C.H], F32, tag="ps_tr")
                nc.tensor.transpose(pt[:], p_sb[:, jt * 128:(jt + 1) * 128],
                                    ident_sb[:C.H, :C.H])
                sm = PL.tile([128, C.H], F32, tag="smaj")
                nc.scalar.activation(sm[:], pt[:], AF.Copy)
                ohg = PO.tile([128, C.NGRAPH], F32, tag="ohg")
                nc.vector.tensor_scalar(ohg[:], iota_sb[:, :C.NGRAPH],
                                        s2g_sb[:, jt:jt + 1], None, AL.is_equal)
                nc.tensor.matmul(ps_g[:], sm[:], ohg[:], start=(jt == 0),
                                 stop=(jt == C.S_T - 1))

            # ---------- head ----------
            g_sb = PL.tile([C.H, C.NGRAPH], F32, tag="gsb")
            nc.scalar.activation(g_sb[:], ps_g[:], AF.Copy)
            ph1 = PSM.tile([C.H, C.NGRAPH], F32, tag="ps_mlp")
            nc.tensor.matmul(ph1[:], l1w_sb[:], g_sb[:], start=True, stop=True)
            t1h = PL.tile([C.H, C.NGRAPH], F32, tag="t1h")
            nc.scalar.activation(t1h[:], ph1[:], AF.Relu, bias=l1b_sb[:])
            ph2 = PSM.tile([C.OUT, C.NGRAPH], F32, tag="ps_mlp")
            nc.tensor.matmul(ph2[:], l2w_sb[:], t1h[:], start=True, stop=True)
            t2h = PL.tile([C.OUT, C.NGRAPH], F32, tag="t2h")
            nc.scalar.activation(t2h[:], ph2[:], AF.Identity, bias=l2b_sb[:])

            # ---------- log_softmax over classes ----------
            ptz = PST.tile([128, C.OUT], F32, tag="ps_tr")
            nc.tensor.transpose(ptz[:C.NGRAPH, :], t2h[:],
                                ident_sb[:C.OUT, :C.OUT])
            z = PL.tile([C.NGRAPH, C.OUT], F32, tag="z")
            nc.scalar.activation(z[:], ptz[:C.NGRAPH, :], AF.Copy)
            mx = PL.tile([C.NGRAPH, 1], F32, tag="mx")
            nc.vector.tensor_reduce(mx[:], z[:], mybir.AxisListType.X, AL.max)
            zc = PL.tile([C.NGRAPH, C.OUT], F32, tag="zc")
            nc.vector.tensor_scalar(zc[:], z[:], mx[:], None, AL.subtract)
            ex = PL.tile([C.NGRAPH, C.OUT], F32, tag="ex")
            nc.scalar.activation(ex[:], zc[:], AF.Exp)
            sm2 = PL.tile([C.NGRAPH, 1], F32, tag="sm2")
            nc.vector.tensor_reduce(sm2[:], ex[:], mybir.AxisListType.X, AL.add)
            ls = PL.tile([C.NGRAPH, 1], F32, tag="ls")
            nc.scalar.activation(ls[:], sm2[:], AF.Ln)
            res = PL.tile([C.NGRAPH, C.OUT], F32, tag="res")
            nc.vector.tensor_scalar(res[:], zc[:], ls[:], None, AL.subtract)
            nc.sync.dma_start(OUTT, res[:])

    nc.compile()
    return nc


# ----------------------------------------------------------------------------
# Runner
# ----------------------------------------------------------------------------

_CACHE = {}


def _run_sim(nc, in_maps, C: Cfg):
    from concourse.bass_interp import MultiCoreSim
    sim = MultiCoreSim(nc, num_cores=C.CORES, trace=False,
                       require_finite=False, require_nnan=False)
    for c in range(C.CORES):
        for k, v in in_maps[c].items():
            sim.cores[c].tensor(k)[:] = v
    sim.simulate(check_with_hw=False)
    return np.array(sim.cores[0].mem_tensor("out"))


def _run_hw(nc, in_maps, C: Cfg, trace=False):
    from concourse.bass_utils import run_bass_kernel_spmd
    res = run_bass_kernel_spmd(nc, in_maps, core_ids=list(range(C.CORES)),
                               trace=trace)
    return res.results[0]["out"], res


def kernel(**inputs):
    C = Cfg()
    in_maps, counts = _prep(inputs, C)
    key = ("full", tuple(counts["NG"]))
    if key not in _CACHE:
        _CACHE[key] = _build(C, counts)
    out, _ = _run_hw(_CACHE[key], in_maps, C)
    return np.asarray(out, np.float32)



# revision 15
# speedup vs baseline: 2.0167x; 2.0167x over previous
"""NestedGIN (4-layer GIN + 2-level pooling + MLP head) on 8 Trainium2 NeuronCores.

Strategy (v2 — streaming bf16 pipeline):
  - Nodes (and their incident in-edges, i.e. edges grouped by dst) are sharded
    across 8 cores; MLP weights are replicated.
  - h lives in HBM as [N, 128] bf16 with each row = [h[i], h[i]] (duplicated)
    so the 256B-row hardware dma_gather delivers bf16 messages directly.
  - Per layer, nodes are processed in 25 "sets" of 512 nodes. Edges are
    host-grouped per (set, src-chunk, 128-node subwindow) into 128-edge tiles
    with uniform (SPMD) tile counts across cores. Scatter-add runs on the
    TensorEngine in bf16: per tile one matmul of the gathered messages against
    a 128-wide one-hot into a statically-placed PSUM column range; one-hots
    are built in bulk on the DVE (one instr per (set,chunk)).
  - The completed PSUM set (the aggregation for 512 nodes) is fused directly
    into the GIN MLP input add (h + agg), so there is no wide agg buffer, no
    dynamic offsets and no register ops.
  - The MLP runs feature-major bf16 ([64, 512] tiles, weights stationary),
    outputs are transposed back to node-major, exported (duplicated) to HBM
    and AllGather'ed for the next layer's gathers.
  - Final: subgraph pooling (one-hot matmul vs node_to_subgraph) -> AllReduce
    of partial subgraph sums -> graph pooling -> MLP head -> log_softmax
    (fp32, as in v1).

Host-side numpy does only index/layout prep (sharding, sorting, padding) plus
the layer-0 input-feature reindex x[src] (pure copy, no arithmetic).
"""

import os
import sys

for _p in ("/opt/trn_rl_repo", "/opt/pypackages"):
    if os.path.isdir(_p) and _p not in sys.path:
        sys.path.append(_p)

import numpy as np

import concourse.bass as bass
import concourse.bacc as bacc
import concourse.tile as tile
import concourse.mybir as mybir

F32 = mybir.dt.float32
BF16 = mybir.dt.bfloat16
I32 = mybir.dt.int32
I16 = mybir.dt.int16


class Cfg:
    def __init__(self):
        self.N, self.E, self.S = 100000, 1600000, 2000
        self.NGRAPH, self.OUT, self.CORES = 64, 8, 8
        self.H = 64
        self.NPC = self.N // self.CORES            # 12500 nodes per core
        self.CHUNK = 25000                          # gather-source chunk (int16)
        self.NCH = self.N // self.CHUNK             # 4 chunks
        self.SET = 512                              # nodes per psum set
        self.NSETS = 25                             # sets per core (24*512+212)
        self.SUB = 128                              # scatter window width
        self.NSUB = 4                               # subwindows per set
        self.NPC_PAD = self.NSETS * self.SET        # 12800
        self.NT_REAL = (self.NPC + 127) // 128      # 98 node-major tiles
        self.LAST_ROWS = self.NPC - (self.NT_REAL - 1) * 128  # 84
        self.SPAD = 2048
        self.SWIN = 512
        self.S_T = self.SPAD // 128


# ----------------------------------------------------------------------------
# Host-side prep: shard + sort edges, build uniform tile schedule, pack inputs.
# ----------------------------------------------------------------------------

def _prep(inputs, C: Cfg):
    x = np.asarray(inputs["x"], np.float32).reshape(C.N)
    ei = np.asarray(inputs["edge_index"]).astype(np.int64)
    n2s = np.asarray(inputs["node_to_subgraph"]).astype(np.int64)
    s2g = np.asarray(inputs["subgraph_to_graph"]).astype(np.int64)
    src_all, dst_all = ei[0], ei[1]

    # -------- per-core edges bucketed by (set, chunk, sub) --------
    # cell_edges[c][s][cv][sub] = (src_local_sorted, off_sorted)
    cell_edges = []
    cnt = np.zeros((C.CORES, C.NSETS, C.NCH, C.NSUB), np.int64)
    for c in range(C.CORES):
        lo = c * C.NPC
        m = (dst_all >= lo) & (dst_all < lo + C.NPC)
        s_, d_ = src_all[m], dst_all[m] - lo
        set_id = d_ // C.SET
        cv_ = s_ // C.CHUNK
        key = (set_id * C.NCH + cv_) * C.SET + (d_ % C.SET)
        o = np.argsort(key, kind="stable")
        s_, d_, set_id, cv_ = s_[o], d_[o], set_id[o], cv_[o]
        sub_ = (d_ % C.SET) // C.SUB
        off_ = d_ % C.SUB
        # counts per cell
        cell_lin = ((set_id * C.NCH + cv_) * C.NSUB + sub_)
        bc = np.bincount(cell_lin, minlength=C.NSETS * C.NCH * C.NSUB)
        cnt[c] = bc.reshape(C.NSETS, C.NCH, C.NSUB)
        cell_edges.append((s_ - cv_ * C.CHUNK, off_, np.cumsum(bc)))

    # uniform slots per cell = max over cores of ceil(cnt/128); ensure every
    # (set, sub) has at least one tile (chunk 0) so psum gets zeroed.
    slots = np.maximum(cnt, 0)
    slots = -(-slots // 128)              # ceil div
    slots = slots.max(axis=0)             # [NSETS, NCH, NSUB] uniform
    for s in range(C.NSETS):
        for sub in range(C.NSUB):
            if slots[s, :, sub].sum() == 0:
                slots[s, 0, sub] = 1
    T_sc = slots.sum(axis=2)              # [NSETS, NCH] tiles per (set,chunk)
    T_set = T_sc.sum(axis=1)              # [NSETS]
    TTOT = int(T_set.sum())
    T_max = int(T_sc.max())
    TS_max = int(T_set.max())

    in_maps = []
    for c in range(C.CORES):
        lo = c * C.NPC
        src_loc, off_arr, ccum = cell_edges[c]
        dstl = np.full((128, TTOT), -1.0, np.float32)
        xsrc = np.zeros((128, TTOT), np.float32)
        idx = np.zeros((16, TTOT * 8), np.int16)
        t_i = 0
        for s in range(C.NSETS):
            for cv in range(C.NCH):
                for sub in range(C.NSUB):
                    cell = (s * C.NCH + cv) * C.NSUB + sub
                    e0 = ccum[cell - 1] if cell > 0 else 0
                    e1 = ccum[cell]
                    n = e1 - e0
                    ns = int(slots[s, cv, sub])
                    cap = ns * 128
                    gl = np.zeros(cap, np.int64)
                    go = np.full(cap, -1.0, np.float32)
                    gl[:n] = src_loc[e0:e1]
                    go[:n] = off_arr[e0:e1]
                    xv = np.zeros(cap, np.float32)
                    xv[:n] = x[src_loc[e0:e1] + cv * C.CHUNK]
                    for k in range(ns):
                        dstl[:, t_i] = go[k * 128:(k + 1) * 128]
                        xsrc[:, t_i] = xv[k * 128:(k + 1) * 128]
                        idx[:, t_i * 8:(t_i + 1) * 8] = \
                            gl[k * 128:(k + 1) * 128].reshape(8, 16).T
                        t_i += 1
        assert t_i == TTOT

        own_n2s = n2s[lo:lo + C.NPC]
        sstart = int(min(max(0, own_n2s.min()), C.SPAD - C.SWIN))
        assert own_n2s.max() - sstart < C.SWIN, "subgraph window overflow"
        n2s_lin = np.full(C.NT_REAL * 128, -1, np.float32)
        n2s_lin[:C.NPC] = own_n2s - sstart
        n2s_loc = n2s_lin.reshape(C.NT_REAL, 128).T

        s2g_lin = np.full(C.SPAD, -1, np.float32)
        s2g_lin[:C.S] = s2g
        s2g_loc = s2g_lin.reshape(C.S_T, 128).T

        xown = np.zeros((1, C.NPC_PAD), np.float32)
        xown[0, :C.NPC] = x[lo:lo + C.NPC]

        import ml_dtypes

        def to_bf16_bits(a):
            return np.ascontiguousarray(
                np.asarray(a, np.float32).astype(ml_dtypes.bfloat16))

        m = {
            "g_idx": np.tile(idx, (8, 1)),
            "g_dstl": to_bf16_bits(dstl),
            "g_xsrc": to_bf16_bits(xsrc),
            "xown": to_bf16_bits(xown),
            "n2s": n2s_loc.astype(np.float32),
            "s2g": s2g_loc.astype(np.float32),
            "sstart": np.array([[sstart]], np.int32),
            "w1a": to_bf16_bits(np.asarray(inputs["conv1_w1"], np.float32)),
            "b1a": np.asarray(inputs["conv1_b1"], np.float32).reshape(C.H, 1),
            "w2a": to_bf16_bits(np.asarray(inputs["conv1_w2"], np.float32)),
            "b2a": np.asarray(inputs["conv1_b2"], np.float32).reshape(C.H, 1),
            "cw1": to_bf16_bits(np.concatenate(list(np.asarray(inputs["convs_w1"], np.float32)), axis=1)),
            "cb1": np.asarray(inputs["convs_b1"], np.float32).T.copy(),
            "cw2": to_bf16_bits(np.concatenate(list(np.asarray(inputs["convs_w2"], np.float32)), axis=1)),
            "cb2": np.asarray(inputs["convs_b2"], np.float32).T.copy(),
            "l1w": np.asarray(inputs["lin1_w"], np.float32),
            "l1b": np.asarray(inputs["lin1_b"], np.float32).reshape(C.H, 1),
            "l2w": np.asarray(inputs["lin2_w"], np.float32),
            "l2b": np.asarray(inputs["lin2_b"], np.float32).reshape(C.OUT, 1),
            "iota": np.tile(np.arange(512, dtype=np.float32), (128, 1)),
            "iotab": to_bf16_bits(np.tile(np.arange(128, dtype=np.float32), (128, 1))),
            "ident": np.eye(128, dtype=np.float32),
            "identb": to_bf16_bits(np.eye(128, dtype=np.float32)),
        }
        in_maps.append(m)

    counts = {
        "slots": slots.tolist(), "T_sc": T_sc.tolist(),
        "T_set": T_set.tolist(), "TTOT": TTOT,
        "T_max": T_max, "TS_max": TS_max,
    }
    return in_maps, counts


# ----------------------------------------------------------------------------
# Device module
# ----------------------------------------------------------------------------

def _build(C: Cfg, counts):
    from concourse.tile import add_dep_helper

    slots = counts["slots"]        # [NSETS][NCH][NSUB]
    T_sc = counts["T_sc"]          # [NSETS][NCH]
    T_set = counts["T_set"]        # [NSETS]
    TTOT = counts["TTOT"]
    T_max = counts["T_max"]
    NL = 4
    AF = mybir.ActivationFunctionType
    AL = mybir.AluOpType

    LOCAL = os.environ.get("KLOCAL") == "1"
    nc = bacc.Bacc("TRN2", target_bir_lowering=False, debug=False,
                   enable_asserts=False, num_swdge_queues=4,
                   num_devices=1 if LOCAL else C.CORES)

    def ein(name, shape, dt=F32):
        return nc.dram_tensor(name, shape, dt, kind="ExternalInput").ap()

    IDX = ein("g_idx", [128, TTOT * 8], I16)
    DSTL = ein("g_dstl", [128, TTOT], BF16)
    XSRC = ein("g_xsrc", [128, TTOT], BF16)
    XOWN = ein("xown", [1, C.NPC_PAD], BF16)
    N2S = ein("n2s", [128, C.NT_REAL])
    S2G = ein("s2g", [128, C.S_T])
    SSTART = ein("sstart", [1, 1], I32)
    W1A, B1A = ein("w1a", [1, C.H], BF16), ein("b1a", [C.H, 1])
    W2A, B2A = ein("w2a", [C.H, C.H], BF16), ein("b2a", [C.H, 1])
    CW1, CB1 = ein("cw1", [C.H, 3 * C.H], BF16), ein("cb1", [C.H, 3])
    CW2, CB2 = ein("cw2", [C.H, 3 * C.H], BF16), ein("cb2", [C.H, 3])
    L1W, L1B = ein("l1w", [C.H, C.H]), ein("l1b", [C.H, 1])
    L2W, L2B = ein("l2w", [C.H, C.OUT]), ein("l2b", [C.OUT, 1])
    IOTA = ein("iota", [128, 512])
    IOTAB = ein("iotab", [128, 128], BF16)
    IDENT = ein("ident", [128, 128])
    IDENTB = ein("identb", [128, 128], BF16)
    OUTT = nc.dram_tensor("out", [C.NGRAPH, C.OUT], F32, kind="ExternalOutput").ap()

    hbuf = [nc.dram_tensor(f"hbuf{k}", [C.N, 128], BF16, kind="Internal",
                           addr_space="Shared").ap() for k in range(2)]
    agin = [nc.dram_tensor(f"agin{k}", [C.NPC, 128], BF16, kind="Internal").ap()
            for k in range(2)]
    ppin = nc.dram_tensor("ppin", [C.H, C.SPAD], F32, kind="Internal").ap()
    ppout = nc.dram_tensor("ppout", [C.H, C.SPAD], F32, kind="Internal",
                           addr_space="Shared").ap()

    RG = [list(range(C.CORES))]

    with tile.TileContext(nc) as tc:
        from concourse import library_config
        nc.gpsimd.load_library(library_config.mlp)
        with (
            tc.tile_pool(name="const", bufs=1) as P0,
            tc.tile_pool(name="mt", bufs=4) as PMT,
            tc.tile_pool(name="oh", bufs=3) as PO,
            tc.tile_pool(name="mlp", bufs=3) as PL,
            tc.tile_pool(name="ps_sc", bufs=2, space="PSUM") as PSC,
            tc.tile_pool(name="ps_mlp", bufs=2, space="PSUM") as PSM,
            tc.tile_pool(name="ps_trb", bufs=2, space="PSUM") as PSTB,
            tc.tile_pool(name="ps_tr", bufs=1, space="PSUM") as PST,
            tc.tile_pool(name="ps_pool", bufs=1, space="PSUM") as PSP,
        ):
            # ---- resident tiles ----
            def load(ap, shape, dt=F32, tag=None):
                t = P0.tile(shape, dt, tag=tag)
                nc.sync.dma_start(t[:], ap)
                return t

            idx_sb = load(IDX, [128, TTOT * 8], I16, tag="c_idx")
            dstl_sb = load(DSTL, [128, TTOT], BF16, tag="c_dstl")
            xsrc_sb = load(XSRC, [128, TTOT], BF16, tag="c_xsrc")
            xown_sb = load(XOWN, [1, C.NPC_PAD], BF16, tag="c_xown")
            n2s_sb = load(N2S, [128, C.NT_REAL], tag="c_n2s")
            s2g_sb = load(S2G, [128, C.S_T], tag="c_s2g")
            sstart_sb = load(SSTART, [1, 1], I32, tag="c_sstart")
            w1a_sb, b1a_sb = load(W1A, [1, C.H], BF16, tag="c_w1a"), load(B1A, [C.H, 1], tag="c_b1a")
            w2a_sb, b2a_sb = load(W2A, [C.H, C.H], BF16, tag="c_w2a"), load(B2A, [C.H, 1], tag="c_b2a")
            cw1_sb, cb1_sb = load(CW1, [C.H, 3 * C.H], BF16, tag="c_cw1"), load(CB1, [C.H, 3], tag="c_cb1")
            cw2_sb, cb2_sb = load(CW2, [C.H, 3 * C.H], BF16, tag="c_cw2"), load(CB2, [C.H, 3], tag="c_cb2")
            l1w_sb, l1b_sb = load(L1W, [C.H, C.H], tag="c_l1w"), load(L1B, [C.H, 1], tag="c_l1b")
            l2w_sb, l2b_sb = load(L2W, [C.H, C.OUT], tag="c_l2w"), load(L2B, [C.OUT, 1], tag="c_l2b")
            iota_sb = load(IOTA, [128, 512], tag="c_iota")
            iotab_sb = load(IOTAB, [128, 128], BF16, tag="c_iotab")
            ident_sb = load(IDENT, [128, 128], tag="c_ident")
            identb_sb = load(IDENTB, [128, 128], BF16, tag="c_identb")

            hT = P0.tile([C.H, C.NPC_PAD], BF16, tag="hT")
            expb = P0.tile([128, C.NT_REAL * C.H], BF16, tag="expb")
            pp_sb = P0.tile([C.H, C.SPAD], F32, tag="pp_sb")
            p_sb = P0.tile([C.H, C.SPAD], F32, tag="p_sb")

            sreg = nc.alloc_registers("sreg", engines=[mybir.EngineType.DVE])

            ag_inst = None

            # precompute per-(set,chunk) tile column offsets
            base_sc = []
            b = 0
            for s in range(C.NSETS):
                row = []
                for cv in range(C.NCH):
                    row.append(b)
                    b += T_sc[s][cv]
                base_sc.append(row)
            assert b == TTOT

            for layer in range(NL):
                l0 = layer == 0
                last = layer == NL - 1
                if l0:
                    w1, b1, w2, b2 = w1a_sb, b1a_sb, w2a_sb, b2a_sb
                else:
                    r0 = (layer - 1) * C.H
                    li = layer - 1
                    w1 = cw1_sb[:, r0:r0 + C.H]
                    b1 = cb1_sb[:, li:li + 1]
                    w2 = cw2_sb[:, r0:r0 + C.H]
                    b2 = cb2_sb[:, li:li + 1]
                src_hbuf = hbuf[(layer + 1) % 2] if not l0 else None

                if last:
                    ps_s = PSP.tile([C.H, C.SWIN], F32, tag="ps_pool")

                for s in range(C.NSETS):
                    sl = slice(s * C.SET, (s + 1) * C.SET)
                    # ONE psum accumulation group per set: start zeroes the
                    # whole 2KB bank (pending-zero), so only the first matmul
                    # of the set starts and only the last stops.
                    n_in_set = sum(slots[s][cv][sub]
                                   for cv in range(C.NCH) for sub in range(C.NSUB))

                    ps = PSC.tile([C.H, C.SET], F32, tag="ps_sc")
                    psr = 1 if l0 else C.H    # rows actually used
                    mm_i = 0
                    for cv in range(C.NCH):
                        tsc = T_sc[s][cv]
                        tb = base_sc[s][cv]
                        if not l0:
                            mt = PMT.tile([128, T_max, 128], BF16, tag="mt")
                            gi = nc.gpsimd.dma_gather(
                                mt[:, 0:tsc, :],
                                src_hbuf[cv * C.CHUNK:(cv + 1) * C.CHUNK, :],
                                idx_sb[:, tb * 8:(tb + tsc) * 8],
                                tsc * 128, tsc * 128, 128,
                                single_packet=False, queue_num=cv)
                            if ag_inst is not None:
                                add_dep_helper(gi.ins, ag_inst.ins,
                                               reason="gather after allgather")
                        oh = PO.tile([128, T_max, 128], BF16, tag="oh")
                        nc.vector.tensor_tensor(
                            oh[:, 0:tsc, :],
                            iotab_sb[:].unsqueeze(1).broadcast_to([128, tsc, 128]),
                            dstl_sb[:, tb:tb + tsc].unsqueeze(2).broadcast_to(
                                [128, tsc, 128]),
                            AL.is_equal)
                        t_local = 0
                        for sub in range(C.NSUB):
                            for k in range(slots[s][cv][sub]):
                                if l0:
                                    lhsT = xsrc_sb[:, tb + t_local:tb + t_local + 1]
                                else:
                                    lhsT = mt[:, t_local, 0:64]
                                nc.tensor.matmul(
                                    ps[0:psr, sub * C.SUB:(sub + 1) * C.SUB],
                                    lhsT, oh[:, t_local, :],
                                    start=(mm_i == 0),
                                    stop=(mm_i == n_in_set - 1))
                                t_local += 1
                                mm_i += 1

                    # ---- fused h + agg -> MLP ----
                    if l0:
                        hin = PL.tile([1, C.SET], BF16, tag="hin0")
                        nc.vector.tensor_tensor(hin[:], xown_sb[0:1, sl],
                                                ps[0:1, :], AL.add)
                    else:
                        hin = PL.tile([C.H, C.SET], BF16, tag="hin")
                        nc.vector.tensor_tensor(hin[:], hT[:, sl], ps[:], AL.add)
                    ps1 = PSM.tile([C.H, C.SET], F32, tag="ps_mlp")
                    nc.tensor.matmul(ps1[:], w1, hin[:], start=True, stop=True)
                    t1 = PL.tile([C.H, C.SET], BF16, tag="t1")
                    nc.scalar.activation(t1[:], ps1[:], AF.Relu, bias=b1)
                    ps2 = PSM.tile([C.H, C.SET], F32, tag="ps_mlp")
                    nc.tensor.matmul(ps2[:], w2, t1[:], start=True, stop=True)
                    nc.scalar.activation(hT[:, sl], ps2[:], AF.Relu, bias=b2)

                    # ---- transpose to node-major (+ pooling on last layer) ----
                    ntile = 4 if s < C.NSETS - 1 else C.NT_REAL - 4 * (C.NSETS - 1)
                    for j in range(ntile):
                        jj = 4 * s + j
                        pt = PSTB.tile([128, C.H], BF16, tag="ps_trb")
                        nc.tensor.transpose(
                            pt[:], hT[:, jj * 128:(jj + 1) * 128],
                            identb_sb[:C.H, :C.H])
                        nc.scalar.activation(expb[:, jj * C.H:(jj + 1) * C.H],
                                             pt[:], AF.Copy)
                        if last:
                            ohp = PO.tile([128, C.SWIN], BF16, tag="ohp")
                            nc.vector.tensor_scalar(
                                ohp[:], iota_sb[:, :C.SWIN],
                                n2s_sb[:, jj:jj + 1], None, AL.is_equal)
                            nc.tensor.matmul(ps_s[:],
                                             expb[:, jj * C.H:(jj + 1) * C.H],
                                             ohp[:], start=(jj == 0),
                                             stop=(jj == C.NT_REAL - 1))

                if not last:
                    # export (duplicated halves) + AllGather
                    dst = agin[layer % 2]
                    nf = C.NT_REAL - 1
                    deps = []
                    for half in range(2):
                        cols = slice(half * 64, half * 64 + 64)
                        d1 = nc.sync.dma_start(
                            dst[0:nf * 128, cols].rearrange("(b p) f -> p b f", p=128),
                            expb[:, :nf * C.H].rearrange("p (b f) -> p b f", f=C.H))
                        d2 = nc.sync.dma_start(
                            dst[nf * 128:C.NPC, cols],
                            expb[:C.LAST_ROWS, nf * C.H:(nf + 1) * C.H])
                        deps += [d1, d2]
                    if LOCAL:
                        ag = nc.sync.dma_start(hbuf[layer % 2][0:C.NPC, :], dst)
                    else:
                        ag = nc.gpsimd.collective_compute(
                            "AllGather", AL.bypass, replica_groups=RG,
                            ins=[dst], outs=[hbuf[layer % 2]])
                    for d in deps:
                        add_dep_helper(ag.ins, d.ins, reason="ag after export")
                    ag_inst = ag

            # ---------- subgraph partial sums -> AllReduce ----------
            nc.vector.memset(pp_sb[:], 0.0)
            nc.vector.reg_load(sreg, sstart_sb[0:1, 0:1])
            soff = nc.vector.snap(sreg, donate=True, min_val=0,
                                  max_val=C.SPAD - C.SWIN)
            dynp = pp_sb[:, bass.ds(soff, C.SWIN)]
            nc.vector.tensor_copy(dynp, ps_s[:])
            d3 = nc.sync.dma_start(ppin, pp_sb[:])
            if LOCAL:
                ar = nc.sync.dma_start(ppout, ppin)
            else:
                ar = nc.gpsimd.collective_compute(
                    "AllReduce", AL.add, replica_groups=RG, ins=[ppin], outs=[ppout])
            add_dep_helper(ar.ins, d3.ins, reason="ar after store")
            d4 = nc.sync.dma_start(p_sb[:], ppout)
            add_dep_helper(d4.ins, ar.ins, reason="load after ar")

            # ---------- graph pooling ----------
            ps_g = PSP.tile([C.H, C.NGRAPH], F32, tag="ps_pool")
            for jt in range(C.S_T):
                pt = PST.tile([128, C.H], F32, tag="ps_tr")
                nc.tensor.transpose(pt[:], p_sb[:, jt * 128:(jt + 1) * 128],
                                    ident_sb[:C.H, :C.H])
                sm = PL.tile([128, C.H], F32, tag="smaj")
                nc.scalar.activation(sm[:], pt[:], AF.Copy)
                ohg = PO.tile([128, C.NGRAPH], F32, tag="ohg")
                nc.vector.tensor_scalar(ohg[:], iota_sb[:, :C.NGRAPH],
                                        s2g_sb[:, jt:jt + 1], None, AL.is_equal)
                nc.tensor.matmul(ps_g[:], sm[:], ohg[:], start=(jt == 0),
                                 stop=(jt == C.S_T - 1))

            # ---------- head ----------
            g_sb = PL.tile([C.H, C.NGRAPH], F32, tag="gsb")
            nc.scalar.activation(g_sb[:], ps_g[:], AF.Copy)
            ph1 = PSM.tile([C.H, C.NGRAPH], F32, tag="ps_mlp")
            nc.tensor.matmul(ph1[:], l1w_sb[:], g_sb[:], start=True, stop=True)
            t1h = PL.tile([C.H, C.NGRAPH], F32, tag="t1h")
            nc.scalar.activation(t1h[:], ph1[:], AF.Relu, bias=l1b_sb[:])
            ph2 = PSM.tile([C.OUT, C.NGRAPH], F32, tag="ps_mlp")
            nc.tensor.matmul(ph2[:], l2w_sb[:], t1h[:], start=True, stop=True)
            t2h = PL.tile([C.OUT, C.NGRAPH], F32, tag="t2h")
            nc.scalar.activation(t2h[:], ph2[:], AF.Identity, bias=l2b_sb[:])

            # ---------- log_softmax over classes ----------
            ptz = PST.tile([128, C.OUT], F32, tag="ps_tr")
            nc.tensor.transpose(ptz[:C.NGRAPH, :], t2h[:],
                                ident_sb[:C.OUT, :C.OUT])
            z = PL.tile([C.NGRAPH, C.OUT], F32, tag="z")
            nc.scalar.activation(z[:], ptz[:C.NGRAPH, :], AF.Copy)
            mx = PL.tile([C.NGRAPH, 1], F32, tag="mx")
            nc.vector.tensor_reduce(mx[:], z[:], mybir.AxisListType.X, AL.max)
            zc = PL.tile([C.NGRAPH, C.OUT], F32, tag="zc")
            nc.vector.tensor_scalar(zc[:], z[:], mx[:], None, AL.subtract)
            ex = PL.tile([C.NGRAPH, C.OUT], F32, tag="ex")
            nc.scalar.activation(ex[:], zc[:], AF.Exp)
            sm2 = PL.tile([C.NGRAPH, 1], F32, tag="sm2")
            nc.vector.tensor_reduce(sm2[:], ex[:], mybir.AxisListType.X, AL.add)
            ls = PL.tile([C.NGRAPH, 1], F32, tag="ls")
            nc.scalar.activation(ls[:], sm2[:], AF.Ln)
            res = PL.tile([C.NGRAPH, C.OUT], F32, tag="res")
            nc.vector.tensor_scalar(res[:], zc[:], ls[:], None, AL.subtract)
            nc.sync.dma_start(OUTT, res[:])

    nc.compile()
    return nc


# ----------------------------------------------------------------------------
# Runner
# ----------------------------------------------------------------------------

_CACHE = {}


def _run_hw(nc, in_maps, C: Cfg, trace=False):
    from concourse.bass_utils import run_bass_kernel_spmd
    res = run_bass_kernel_spmd(nc, in_maps, core_ids=list(range(C.CORES)),
                               trace=trace)
    return res.results[0]["out"], res


def kernel(**inputs):
    C = Cfg()
    in_maps, counts = _prep(inputs, C)
    key = ("v2", counts["TTOT"], tuple(counts["T_set"]))
    if key not in _CACHE:
        _CACHE[key] = _build(C, counts)
    out, _ = _run_hw(_CACHE[key], in_maps, C)
    return np.asarray(out, np.float32)


# revision 25
# speedup vs baseline: 2.6867x; 1.3323x over previous
"""NestedGIN (4-layer GIN + 2-level pooling + MLP head) on 8 Trainium2 NeuronCores.

Strategy (v2 — streaming bf16 pipeline):
  - Nodes (and their incident in-edges, i.e. edges grouped by dst) are sharded
    across 8 cores; MLP weights are replicated.
  - h lives in HBM as [N, 128] bf16 with each row = [h[i], h[i]] (duplicated)
    so the 256B-row hardware dma_gather delivers bf16 messages directly.
  - Per layer, nodes are processed in 25 "sets" of 512 nodes. Edges are
    host-grouped per (set, src-chunk, 128-node subwindow) into 128-edge tiles
    with uniform (SPMD) tile counts across cores. Scatter-add runs on the
    TensorEngine in bf16: per tile one matmul of the gathered messages against
    a 128-wide one-hot into a statically-placed PSUM column range; one-hots
    are built in bulk on the DVE (one instr per (set,chunk)).
  - The completed PSUM set (the aggregation for 512 nodes) is fused directly
    into the GIN MLP input add (h + agg), so there is no wide agg buffer, no
    dynamic offsets and no register ops.
  - The MLP runs feature-major bf16 ([64, 512] tiles, weights stationary),
    outputs are transposed back to node-major, exported (duplicated) to HBM
    and AllGather'ed for the next layer's gathers.
  - Final: subgraph pooling (one-hot matmul vs node_to_subgraph) -> AllReduce
    of partial subgraph sums -> graph pooling -> MLP head -> log_softmax
    (fp32, as in v1).

Host-side numpy does only index/layout prep (sharding, sorting, padding) plus
the layer-0 input-feature reindex x[src] (pure copy, no arithmetic).
"""

import os
import sys

for _p in ("/opt/trn_rl_repo", "/opt/pypackages"):
    if os.path.isdir(_p) and _p not in sys.path:
        sys.path.append(_p)

import numpy as np

import concourse.bass as bass
import concourse.bacc as bacc
import concourse.tile as tile
import concourse.mybir as mybir

F32 = mybir.dt.float32
BF16 = mybir.dt.bfloat16
I32 = mybir.dt.int32
I16 = mybir.dt.int16


class Cfg:
    def __init__(self):
        self.N, self.E, self.S = 100000, 1600000, 2000
        self.NGRAPH, self.OUT, self.CORES = 64, 8, 8
        self.H = 64
        self.NPC = self.N // self.CORES            # 12500 nodes per core
        self.CHUNK = 25000                          # gather-source chunk (int16)
        self.NCH = self.N // self.CHUNK             # 4 chunks
        self.SET = 512                              # nodes per psum set
        self.NSETS = 25                             # sets per core (24*512+212)
        self.SUB = 128                              # scatter window width
        self.NSUB = 4                               # subwindows per set
        self.NPC_PAD = self.NSETS * self.SET        # 12800
        self.NT_REAL = (self.NPC + 127) // 128      # 98 node-major tiles
        self.LAST_ROWS = self.NPC - (self.NT_REAL - 1) * 128  # 84
        self.SPAD = 2048
        self.SWIN = 512
        self.S_T = self.SPAD // 128


# ----------------------------------------------------------------------------
# Host-side prep: shard + sort edges, build uniform tile schedule, pack inputs.
# ----------------------------------------------------------------------------

def _prep(inputs, C: Cfg):
    x = np.asarray(inputs["x"], np.float32).reshape(C.N)
    ei = np.asarray(inputs["edge_index"]).astype(np.int64)
    n2s = np.asarray(inputs["node_to_subgraph"]).astype(np.int64)
    s2g = np.asarray(inputs["subgraph_to_graph"]).astype(np.int64)
    src_all, dst_all = ei[0], ei[1]

    # -------- per-core edges bucketed by (set, chunk, sub) --------
    # cell_edges[c][s][cv][sub] = (src_local_sorted, off_sorted)
    cell_edges = []
    cnt = np.zeros((C.CORES, C.NSETS, C.NCH, C.NSUB), np.int64)
    for c in range(C.CORES):
        lo = c * C.NPC
        m = (dst_all >= lo) & (dst_all < lo + C.NPC)
        s_, d_ = src_all[m], dst_all[m] - lo
        set_id = d_ // C.SET
        cv_ = s_ // C.CHUNK
        key = (set_id * C.NCH + cv_) * C.SET + (d_ % C.SET)
        o = np.argsort(key, kind="stable")
        s_, d_, set_id, cv_ = s_[o], d_[o], set_id[o], cv_[o]
        sub_ = (d_ % C.SET) // C.SUB
        off_ = d_ % C.SUB
        # counts per cell
        cell_lin = ((set_id * C.NCH + cv_) * C.NSUB + sub_)
        bc = np.bincount(cell_lin, minlength=C.NSETS * C.NCH * C.NSUB)
        cnt[c] = bc.reshape(C.NSETS, C.NCH, C.NSUB)
        cell_edges.append((s_ - cv_ * C.CHUNK, off_, np.cumsum(bc)))

    # uniform slots per cell = max over cores of ceil(cnt/128); ensure every
    # (set, sub) has at least one tile (chunk 0) so psum gets zeroed.
    slots = np.maximum(cnt, 0)
    slots = -(-slots // 128)              # ceil div
    slots = slots.max(axis=0)             # [NSETS, NCH, NSUB] uniform
    for s in range(C.NSETS):
        for sub in range(C.NSUB):
            if slots[s, :, sub].sum() == 0:
                slots[s, 0, sub] = 1
    T_sc = slots.sum(axis=2)              # [NSETS, NCH] tiles per (set,chunk)
    T_set = T_sc.sum(axis=1)              # [NSETS]
    TTOT = int(T_set.sum())
    T_max = int(T_sc.max())
    TS_max = int(T_set.max())

    in_maps = []
    for c in range(C.CORES):
        lo = c * C.NPC
        src_loc, off_arr, ccum = cell_edges[c]
        dstl = np.full((128, TTOT), -1.0, np.float32)
        xsrc = np.zeros((128, TTOT), np.float32)
        idx = np.zeros((16, TTOT * 8), np.int16)
        t_i = 0
        for s in range(C.NSETS):
            for cv in range(C.NCH):
                for sub in range(C.NSUB):
                    cell = (s * C.NCH + cv) * C.NSUB + sub
                    e0 = ccum[cell - 1] if cell > 0 else 0
                    e1 = ccum[cell]
                    n = e1 - e0
                    ns = int(slots[s, cv, sub])
                    cap = ns * 128
                    # sort the cell's edges by src for HBM locality in the
                    # gather (the (idx, dst-offset) pairing is preserved)
                    so = np.argsort(src_loc[e0:e1], kind="stable")
                    gl = np.zeros(cap, np.int64)
                    go = np.full(cap, -1.0, np.float32)
                    gl[:n] = src_loc[e0:e1][so]
                    go[:n] = off_arr[e0:e1][so]
                    xv = np.zeros(cap, np.float32)
                    xv[:n] = x[gl[:n] + cv * C.CHUNK]
                    for k in range(ns):
                        dstl[:, t_i] = go[k * 128:(k + 1) * 128]
                        xsrc[:, t_i] = xv[k * 128:(k + 1) * 128]
                        idx[:, t_i * 8:(t_i + 1) * 8] = \
                            gl[k * 128:(k + 1) * 128].reshape(8, 16).T
                        t_i += 1
        assert t_i == TTOT

        own_n2s = n2s[lo:lo + C.NPC]
        sstart = int(min(max(0, own_n2s.min()), C.SPAD - C.SWIN))
        assert own_n2s.max() - sstart < C.SWIN, "subgraph window overflow"
        n2s_lin = np.full(C.NT_REAL * 128, -1, np.float32)
        n2s_lin[:C.NPC] = own_n2s - sstart
        n2s_loc = n2s_lin.reshape(C.NT_REAL, 128).T

        s2g_lin = np.full(C.SPAD, -1, np.float32)
        s2g_lin[:C.S] = s2g
        s2g_loc = s2g_lin.reshape(C.S_T, 128).T

        xown = np.zeros((1, C.NPC_PAD), np.float32)
        xown[0, :C.NPC] = x[lo:lo + C.NPC]

        import ml_dtypes

        def to_bf16_bits(a):
            return np.ascontiguousarray(
                np.asarray(a, np.float32).astype(ml_dtypes.bfloat16))

        m = {
            "g_idx": np.tile(idx, (8, 1)),
            "g_dstl": to_bf16_bits(np.repeat(dstl, 2, axis=1)),
            "g_xsrc": to_bf16_bits(xsrc),
            "xown": to_bf16_bits(xown),
            "n2s": n2s_loc.astype(np.float32),
            "s2g": s2g_loc.astype(np.float32),
            "sstart": np.array([[sstart]], np.int32),
            "w1a": to_bf16_bits(np.asarray(inputs["conv1_w1"], np.float32)),
            "b1a": np.asarray(inputs["conv1_b1"], np.float32).reshape(C.H, 1),
            "w2a": to_bf16_bits(np.asarray(inputs["conv1_w2"], np.float32)),
            "b2a": np.asarray(inputs["conv1_b2"], np.float32).reshape(C.H, 1),
            "cw1": to_bf16_bits(np.concatenate(list(np.asarray(inputs["convs_w1"], np.float32)), axis=1)),
            "cb1": np.asarray(inputs["convs_b1"], np.float32).T.copy(),
            "cw2": to_bf16_bits(np.concatenate(list(np.asarray(inputs["convs_w2"], np.float32)), axis=1)),
            "cb2": np.asarray(inputs["convs_b2"], np.float32).T.copy(),
            "l1w": np.asarray(inputs["lin1_w"], np.float32),
            "l1b": np.asarray(inputs["lin1_b"], np.float32).reshape(C.H, 1),
            "l2w": np.asarray(inputs["lin2_w"], np.float32),
            "l2b": np.asarray(inputs["lin2_b"], np.float32).reshape(C.OUT, 1),
            "iota": np.tile(np.arange(512, dtype=np.float32), (128, 1)),
            "iotab": to_bf16_bits(np.tile(np.arange(128, dtype=np.float32), (128, 1))),
            "ident": np.eye(128, dtype=np.float32),
            "identb": to_bf16_bits(np.eye(128, dtype=np.float32)),
        }
        in_maps.append(m)

    counts = {
        "slots": slots.tolist(), "T_sc": T_sc.tolist(),
        "T_set": T_set.tolist(), "TTOT": TTOT,
        "T_max": T_max, "TS_max": TS_max,
    }
    return in_maps, counts


# ----------------------------------------------------------------------------
# Device module
# ----------------------------------------------------------------------------

def _build(C: Cfg, counts):
    from concourse.tile import add_dep_helper

    slots = counts["slots"]        # [NSETS][NCH][NSUB]
    T_sc = counts["T_sc"]          # [NSETS][NCH]
    T_set = counts["T_set"]        # [NSETS]
    TTOT = counts["TTOT"]
    T_max = counts["T_max"]
    NL = 4
    AF = mybir.ActivationFunctionType
    AL = mybir.AluOpType

    LOCAL = os.environ.get("KLOCAL") == "1"
    nc = bacc.Bacc("TRN2", target_bir_lowering=False, debug=False,
                   enable_asserts=False, num_swdge_queues=4,
                   num_devices=1 if LOCAL else C.CORES)

    def ein(name, shape, dt=F32):
        return nc.dram_tensor(name, shape, dt, kind="ExternalInput").ap()

    IDX = ein("g_idx", [128, TTOT * 8], I16)
    DSTL = ein("g_dstl", [128, TTOT * 2], BF16)
    XSRC = ein("g_xsrc", [128, TTOT], BF16)
    XOWN = ein("xown", [1, C.NPC_PAD], BF16)
    N2S = ein("n2s", [128, C.NT_REAL])
    S2G = ein("s2g", [128, C.S_T])
    SSTART = ein("sstart", [1, 1], I32)
    W1A, B1A = ein("w1a", [1, C.H], BF16), ein("b1a", [C.H, 1])
    W2A, B2A = ein("w2a", [C.H, C.H], BF16), ein("b2a", [C.H, 1])
    CW1, CB1 = ein("cw1", [C.H, 3 * C.H], BF16), ein("cb1", [C.H, 3])
    CW2, CB2 = ein("cw2", [C.H, 3 * C.H], BF16), ein("cb2", [C.H, 3])
    L1W, L1B = ein("l1w", [C.H, C.H]), ein("l1b", [C.H, 1])
    L2W, L2B = ein("l2w", [C.H, C.OUT]), ein("l2b", [C.OUT, 1])
    IOTA = ein("iota", [128, 512])
    IOTAB = ein("iotab", [128, 128], BF16)
    IDENT = ein("ident", [128, 128])
    IDENTB = ein("identb", [128, 128], BF16)
    OUTT = nc.dram_tensor("out", [C.NGRAPH, C.OUT], F32, kind="ExternalOutput").ap()

    hbuf = [nc.dram_tensor(f"hbuf{k}", [C.N, 128], BF16, kind="Internal",
                           addr_space="Shared").ap() for k in range(2)]
    agin = [nc.dram_tensor(f"agin{k}", [C.NPC, 128], BF16, kind="Internal").ap()
            for k in range(2)]
    ppin = nc.dram_tensor("ppin", [C.H, C.SPAD], F32, kind="Internal").ap()
    ppout = nc.dram_tensor("ppout", [C.H, C.SPAD], F32, kind="Internal",
                           addr_space="Shared").ap()

    RG = [list(range(C.CORES))]

    with tile.TileContext(nc) as tc:
        from concourse import library_config
        nc.gpsimd.load_library(library_config.mlp)
        with (
            tc.tile_pool(name="const", bufs=1) as P0,
            tc.tile_pool(name="mt", bufs=6) as PMT,
            tc.tile_pool(name="oh", bufs=4) as PO,
            tc.tile_pool(name="mlp", bufs=3) as PL,
            tc.tile_pool(name="ps_sc", bufs=2, space="PSUM") as PSC,
            tc.tile_pool(name="ps_mlp", bufs=2, space="PSUM") as PSM,
            tc.tile_pool(name="ps_trb", bufs=2, space="PSUM") as PSTB,
            tc.tile_pool(name="ps_tr", bufs=1, space="PSUM") as PST,
            tc.tile_pool(name="ps_pool", bufs=1, space="PSUM") as PSP,
        ):
            # ---- resident tiles ----
            def load(ap, shape, dt=F32, tag=None):
                t = P0.tile(shape, dt, tag=tag)
                nc.sync.dma_start(t[:], ap)
                return t

            idx_sb = load(IDX, [128, TTOT * 8], I16, tag="c_idx")
            dstl_sb = load(DSTL, [128, TTOT * 2], BF16, tag="c_dstl")
            xsrc_sb = load(XSRC, [128, TTOT], BF16, tag="c_xsrc")
            xown_sb = load(XOWN, [1, C.NPC_PAD], BF16, tag="c_xown")
            n2s_sb = load(N2S, [128, C.NT_REAL], tag="c_n2s")
            s2g_sb = load(S2G, [128, C.S_T], tag="c_s2g")
            sstart_sb = load(SSTART, [1, 1], I32, tag="c_sstart")
            w1a_sb, b1a_sb = load(W1A, [1, C.H], BF16, tag="c_w1a"), load(B1A, [C.H, 1], tag="c_b1a")
            w2a_sb, b2a_sb = load(W2A, [C.H, C.H], BF16, tag="c_w2a"), load(B2A, [C.H, 1], tag="c_b2a")
            cw1_sb, cb1_sb = load(CW1, [C.H, 3 * C.H], BF16, tag="c_cw1"), load(CB1, [C.H, 3], tag="c_cb1")
            cw2_sb, cb2_sb = load(CW2, [C.H, 3 * C.H], BF16, tag="c_cw2"), load(CB2, [C.H, 3], tag="c_cb2")
            l1w_sb, l1b_sb = load(L1W, [C.H, C.H], tag="c_l1w"), load(L1B, [C.H, 1], tag="c_l1b")
            l2w_sb, l2b_sb = load(L2W, [C.H, C.OUT], tag="c_l2w"), load(L2B, [C.OUT, 1], tag="c_l2b")
            iota_sb = load(IOTA, [128, 512], tag="c_iota")
            iotab_sb = load(IOTAB, [128, 128], BF16, tag="c_iotab")
            ident_sb = load(IDENT, [128, 128], tag="c_ident")
            identb_sb = load(IDENTB, [128, 128], BF16, tag="c_identb")

            hT = P0.tile([C.H, C.NPC_PAD], BF16, tag="hT")
            expb = P0.tile([128, C.NT_REAL * C.H], BF16, tag="expb")
            pp_sb = P0.tile([C.H, C.SPAD], F32, tag="pp_sb")
            p_sb = P0.tile([C.H, C.SPAD], F32, tag="p_sb")

            sreg = nc.alloc_registers("sreg", engines=[mybir.EngineType.DVE])

            ag_inst = None

            # precompute per-(set,chunk) tile column offsets
            base_sc = []
            b = 0
            for s in range(C.NSETS):
                row = []
                for cv in range(C.NCH):
                    row.append(b)
                    b += T_sc[s][cv]
                base_sc.append(row)
            assert b == TTOT

            for layer in range(NL):
                l0 = layer == 0
                last = layer == NL - 1
                if l0:
                    w1, b1, w2, b2 = w1a_sb, b1a_sb, w2a_sb, b2a_sb
                else:
                    r0 = (layer - 1) * C.H
                    li = layer - 1
                    w1 = cw1_sb[:, r0:r0 + C.H]
                    b1 = cb1_sb[:, li:li + 1]
                    w2 = cw2_sb[:, r0:r0 + C.H]
                    b2 = cb2_sb[:, li:li + 1]
                src_hbuf = hbuf[(layer + 1) % 2] if not l0 else None

                if last:
                    ps_s = PSP.tile([C.H, C.SWIN], F32, tag="ps_pool")

                for s in range(C.NSETS):
                    sl = slice(s * C.SET, (s + 1) * C.SET)
                    # ONE psum accumulation group per set: start zeroes the
                    # whole 2KB bank (pending-zero), so only the first matmul
                    # of the set starts and only the last stops.
                    n_in_set = sum(slots[s][cv][sub]
                                   for cv in range(C.NCH) for sub in range(C.NSUB))

                    ps = PSC.tile([C.H, C.SET], F32, tag="ps_sc")
                    psr = 1 if l0 else C.H    # rows actually used
                    mm_i = 0
                    for cv in range(C.NCH):
                        tsc = T_sc[s][cv]
                        tb = base_sc[s][cv]
                        if not l0:
                            mt = PMT.tile([128, T_max, 128], BF16, tag="mt")
                            gi = nc.gpsimd.dma_gather(
                                mt[:, 0:tsc, :],
                                src_hbuf[cv * C.CHUNK:(cv + 1) * C.CHUNK, :],
                                idx_sb[:, tb * 8:(tb + tsc) * 8],
                                tsc * 128, tsc * 128, 128,
                                single_packet=False, queue_num=cv)
                            if ag_inst is not None:
                                add_dep_helper(gi.ins, ag_inst.ins,
                                               reason="gather after allgather")
                        oh = PO.tile([128, T_max, 128], BF16, tag="oh")
                        nc.vector.tensor_tensor(
                            oh[:, 0:tsc, :],
                            iotab_sb[:].unsqueeze(1).broadcast_to([128, tsc, 128]),
                            dstl_sb[:, tb * 2:(tb + tsc) * 2]
                            .rearrange("p (t b) -> p t b", b=2)[:, :, 0:1]
                            .broadcast_to([128, tsc, 128]),
                            AL.is_equal)
                        t_local = 0
                        for sub in range(C.NSUB):
                            for k in range(slots[s][cv][sub]):
                                if l0:
                                    lhsT = xsrc_sb[:, tb + t_local:tb + t_local + 1]
                                else:
                                    lhsT = mt[:, t_local, 0:64]
                                nc.tensor.matmul(
                                    ps[0:psr, sub * C.SUB:(sub + 1) * C.SUB],
                                    lhsT, oh[:, t_local, :],
                                    start=(mm_i == 0), stop=False)
                                t_local += 1
                                mm_i += 1

                    # fold "+ h" into the psum via identity accumulate (PE)
                    if l0:
                        nc.tensor.matmul(ps[0:1, :], identb_sb[0:1, 0:1],
                                         xown_sb[0:1, sl], start=False, stop=True)
                    else:
                        nc.tensor.matmul(ps[:], identb_sb[:C.H, :C.H],
                                         hT[:, sl], start=False, stop=True)

                    # ---- h + agg (already summed in psum) -> MLP ----
                    if l0:
                        hin = PL.tile([1, C.SET], BF16, tag="hin0")
                        nc.scalar.activation(hin[:], ps[0:1, :], AF.Copy)
                    else:
                        hin = PL.tile([C.H, C.SET], BF16, tag="hin")
                        nc.scalar.activation(hin[:], ps[:], AF.Copy)
                    ps1 = PSM.tile([C.H, C.SET], F32, tag="ps_mlp")
                    nc.tensor.matmul(ps1[:], w1, hin[:], start=True, stop=True)
                    t1 = PL.tile([C.H, C.SET], BF16, tag="t1")
                    nc.scalar.activation(t1[:], ps1[:], AF.Relu, bias=b1)
                    ps2 = PSM.tile([C.H, C.SET], F32, tag="ps_mlp")
                    nc.tensor.matmul(ps2[:], w2, t1[:], start=True, stop=True)
                    nc.scalar.activation(hT[:, sl], ps2[:], AF.Relu, bias=b2)

                    # ---- transpose to node-major (+ pooling on last layer) ----
                    ntile = 4 if s < C.NSETS - 1 else C.NT_REAL - 4 * (C.NSETS - 1)
                    for j in range(ntile):
                        jj = 4 * s + j
                        pt = PSTB.tile([128, C.H], BF16, tag="ps_trb")
                        nc.tensor.transpose(
                            pt[:], hT[:, jj * 128:(jj + 1) * 128],
                            identb_sb[:C.H, :C.H])
                        nc.scalar.activation(expb[:, jj * C.H:(jj + 1) * C.H],
                                             pt[:], AF.Copy)
                        if last:
                            ohp = PO.tile([128, C.SWIN], BF16, tag="ohp")
                            nc.vector.tensor_scalar(
                                ohp[:], iota_sb[:, :C.SWIN],
                                n2s_sb[:, jj:jj + 1], None, AL.is_equal)
                            nc.tensor.matmul(ps_s[:],
                                             expb[:, jj * C.H:(jj + 1) * C.H],
                                             ohp[:], start=(jj == 0),
                                             stop=(jj == C.NT_REAL - 1))

                if not last:
                    # export (duplicated halves) + AllGather
                    dst = agin[layer % 2]
                    nf = C.NT_REAL - 1
                    deps = []
                    for half in range(2):
                        cols = slice(half * 64, half * 64 + 64)
                        d1 = nc.sync.dma_start(
                            dst[0:nf * 128, cols].rearrange("(b p) f -> p b f", p=128),
                            expb[:, :nf * C.H].rearrange("p (b f) -> p b f", f=C.H))
                        d2 = nc.sync.dma_start(
                            dst[nf * 128:C.NPC, cols],
                            expb[:C.LAST_ROWS, nf * C.H:(nf + 1) * C.H])
                        deps += [d1, d2]
                    if LOCAL:
                        ag = nc.sync.dma_start(hbuf[layer % 2][0:C.NPC, :], dst)
                    else:
                        ag = nc.gpsimd.collective_compute(
                            "AllGather", AL.bypass, replica_groups=RG,
                            ins=[dst], outs=[hbuf[layer % 2]])
                    for d in deps:
                        add_dep_helper(ag.ins, d.ins, reason="ag after export")
                    ag_inst = ag

            # ---------- subgraph partial sums -> AllReduce ----------
            nc.vector.memset(pp_sb[:], 0.0)
            nc.vector.reg_load(sreg, sstart_sb[0:1, 0:1])
            soff = nc.vector.snap(sreg, donate=True, min_val=0,
                                  max_val=C.SPAD - C.SWIN)
            dynp = pp_sb[:, bass.ds(soff, C.SWIN)]
            nc.vector.tensor_copy(dynp, ps_s[:])
            d3 = nc.sync.dma_start(ppin, pp_sb[:])
            if LOCAL:
                ar = nc.sync.dma_start(ppout, ppin)
            else:
                ar = nc.gpsimd.collective_compute(
                    "AllReduce", AL.add, replica_groups=RG, ins=[ppin], outs=[ppout])
            add_dep_helper(ar.ins, d3.ins, reason="ar after store")
            d4 = nc.sync.dma_start(p_sb[:], ppout)
            add_dep_helper(d4.ins, ar.ins, reason="load after ar")

            # ---------- graph pooling ----------
            ps_g = PSP.tile([C.H, C.NGRAPH], F32, tag="ps_pool")
            for jt in range(C.S_T):
                pt = PST.tile([128, C.H], F32, tag="ps_tr")
                nc.tensor.transpose(pt[:], p_sb[:, jt * 128:(jt + 1) * 128],
                                    ident_sb[:C.H, :C.H])
                sm = PL.tile([128, C.H], F32, tag="smaj")
                nc.scalar.activation(sm[:], pt[:], AF.Copy)
                ohg = PO.tile([128, C.NGRAPH], F32, tag="ohg")
                nc.vector.tensor_scalar(ohg[:], iota_sb[:, :C.NGRAPH],
                                        s2g_sb[:, jt:jt + 1], None, AL.is_equal)
                nc.tensor.matmul(ps_g[:], sm[:], ohg[:], start=(jt == 0),
                                 stop=(jt == C.S_T - 1))

            # ---------- head ----------
            g_sb = PL.tile([C.H, C.NGRAPH], F32, tag="gsb")
            nc.scalar.activation(g_sb[:], ps_g[:], AF.Copy)
            ph1 = PSM.tile([C.H, C.NGRAPH], F32, tag="ps_mlp")
            nc.tensor.matmul(ph1[:], l1w_sb[:], g_sb[:], start=True, stop=True)
            t1h = PL.tile([C.H, C.NGRAPH], F32, tag="t1h")
            nc.scalar.activation(t1h[:], ph1[:], AF.Relu, bias=l1b_sb[:])
            ph2 = PSM.tile([C.OUT, C.NGRAPH], F32, tag="ps_mlp")
            nc.tensor.matmul(ph2[:], l2w_sb[:], t1h[:], start=True, stop=True)
            t2h = PL.tile([C.OUT, C.NGRAPH], F32, tag="t2h")
            nc.scalar.activation(t2h[:], ph2[:], AF.Identity, bias=l2b_sb[:])

            # ---------- log_softmax over classes ----------
            ptz = PST.tile([128, C.OUT], F32, tag="ps_tr")
            nc.tensor.transpose(ptz[:C.NGRAPH, :], t2h[:],
                                ident_sb[:C.OUT, :C.OUT])
            z = PL.tile([C.NGRAPH, C.OUT], F32, tag="z")
            nc.scalar.activation(z[:], ptz[:C.NGRAPH, :], AF.Copy)
            mx = PL.tile([C.NGRAPH, 1], F32, tag="mx")
            nc.vector.tensor_reduce(mx[:], z[:], mybir.AxisListType.X, AL.max)
            zc = PL.tile([C.NGRAPH, C.OUT], F32, tag="zc")
            nc.vector.tensor_scalar(zc[:], z[:], mx[:], None, AL.subtract)
            ex = PL.tile([C.NGRAPH, C.OUT], F32, tag="ex")
            nc.scalar.activation(ex[:], zc[:], AF.Exp)
            sm2 = PL.tile([C.NGRAPH, 1], F32, tag="sm2")
            nc.vector.tensor_reduce(sm2[:], ex[:], mybir.AxisListType.X, AL.add)
            ls = PL.tile([C.NGRAPH, 1], F32, tag="ls")
            nc.scalar.activation(ls[:], sm2[:], AF.Ln)
            res = PL.tile([C.NGRAPH, C.OUT], F32, tag="res")
            nc.vector.tensor_scalar(res[:], zc[:], ls[:], None, AL.subtract)
            nc.sync.dma_start(OUTT, res[:])

    nc.compile()
    return nc


# ----------------------------------------------------------------------------
# Runner
# ----------------------------------------------------------------------------

_CACHE = {}


def _run_hw(nc, in_maps, C: Cfg, trace=False):
    from concourse.bass_utils import run_bass_kernel_spmd
    res = run_bass_kernel_spmd(nc, in_maps, core_ids=list(range(C.CORES)),
                               trace=trace)
    return res.results[0]["out"], res


def kernel(**inputs):
    C = Cfg()
    in_maps, counts = _prep(inputs, C)
    key = ("v2", counts["TTOT"], tuple(counts["T_set"]))
    if key not in _CACHE:
        _CACHE[key] = _build(C, counts)
    out, _ = _run_hw(_CACHE[key], in_maps, C)
    return np.asarray(out, np.float32)
